# revision 7
# baseline (speedup 1.0000x reference)
"""Trainium2 Bass kernel for nn_BasicModule_43911745634888 (sparse_attention).

Computation (see reference):
  scores = einsum('sbh,bh->bs', ctx, key) + mask ; At = softmax(scores, axis=S)
  attn   = einsum('bs,sbh->bh', At, ctx)
  x1     = concat([attn, key], 1) ; y = x1 @ fc1_w.T (+fc1_b, cancelled by BN)
  BN(train stats over batch) -> tanh -> z = t @ fc2_w.T + fc2_b
  returns (z, At)

Sharding: 8 NeuronCores.
  - Attention: data-parallel over batch (8 batches/core). Single pass over the
    core's 64 MB ctx slice: fused DVE multiply+accumulate for scores, per-tile
    softmax max, fp16 PE matmuls for the weighted sum (partials rescaled by
    exp(m_t - M) at the end).
  - MLP head: feature-parallel (128 of 1024 fc1 output features per core).
    attn^T is AllGather'ed (32 KB/core), each core computes its 128 features
    of y for all 64 batches -> BN stats are local -> tanh -> partial fc2 ->
    ReduceScatter(add) hands each core its 8-batch slice of z.
"""

import numpy as np

N_CORES = 8
S, B, H = 2048, 64, 1024
BL = B // N_CORES          # batches per core = 8
OL = H // N_CORES          # fc1 output features per core = 128
NT = S // 128              # s-tiles per core = 16
P = 128
F = BL * H                 # flattened (b, h) free size = 8192
NC_CHUNKS = F // P         # 64 chunk matmuls per tile
BN_EPS = 1e-5

_cache = {}


def _build():
    import concourse.bass as bass
    import concourse.bacc as bacc
    import concourse.tile as tile
    import concourse.mybir as mybir
    from concourse.masks import make_identity
    from contextlib import ExitStack

    fp32 = mybir.dt.float32
    fp16 = mybir.dt.float16
    Alu = mybir.AluOpType
    Act = mybir.ActivationFunctionType

    nc = bacc.Bacc("TRN2", target_bir_lowering=False, debug=False,
                   num_devices=N_CORES)

    # ---- per-core DRAM I/O ----
    ctx_s = nc.dram_tensor("ctx_s", [S, BL, H], fp32, kind="ExternalInput").ap()
    key_l = nc.dram_tensor("key_l", [BL, H], fp32, kind="ExternalInput").ap()
    keyT_f = nc.dram_tensor("keyT_f", [H, B], fp32, kind="ExternalInput").ap()
    maskT_s = nc.dram_tensor("maskT_s", [S, BL], fp32, kind="ExternalInput").ap()
    w1s = nc.dram_tensor("w1s", [2 * H, OL], fp32, kind="ExternalInput").ap()
    w2s = nc.dram_tensor("w2s", [OL, H], fp32, kind="ExternalInput").ap()
    gam = nc.dram_tensor("gam", [OL, 1], fp32, kind="ExternalInput").ap()
    bet = nc.dram_tensor("bet", [OL, 1], fp32, kind="ExternalInput").ap()
    fc2b = nc.dram_tensor("fc2b", [1, H], fp32, kind="ExternalInput").ap()
    z_out = nc.dram_tensor("z_out", [BL, H], fp32, kind="ExternalOutput").ap()
    at_out = nc.dram_tensor("at_out", [BL, S], fp32, kind="ExternalOutput").ap()

    with tile.TileContext(nc) as tc:
        with ExitStack() as est:
            sing = est.enter_context(tc.tile_pool(name="sing", bufs=1))
            dram = est.enter_context(tc.tile_pool(name="dram", bufs=1, space="DRAM"))

            # ---- prologue: constants ----
            key_1p = sing.tile([1, BL, H], fp32)
            nc.sync.dma_start(out=key_1p[:], in_=key_l.rearrange("(o b) h -> o b h", o=1))
            key_rep = sing.tile([P, BL, H], fp32)
            nc.gpsimd.partition_broadcast(key_rep[:], key_1p[:])

            maskT = sing.tile([P, NT, BL], fp32)
            nc.sync.dma_start(out=maskT[:], in_=maskT_s.rearrange("(t p) b -> p t b", p=P))

            ident = sing.tile([P, P], fp32)
            make_identity(nc, ident[:])
            ones1 = sing.tile([1, P], fp32)
            nc.vector.memset(ones1[:], 1.0)
            eps_sb = sing.tile([P, 1], fp32)
            nc.vector.memset(eps_sb[:], BN_EPS)

            w1sb = sing.tile([P, 2 * H // P, OL], fp32)   # (128, 16, 128) k-chunks
            nc.sync.dma_start(out=w1sb[:], in_=w1s.rearrange("(kc p) o -> p kc o", p=P))
            w2sb = sing.tile([OL, H], fp32)
            nc.sync.dma_start(out=w2sb[:], in_=w2s)
            keyT_sb = sing.tile([P, H // P, B], fp32)  # (128, 8, 64)
            nc.sync.dma_start(out=keyT_sb[:], in_=keyT_f.rearrange("(kc p) b -> p kc b", p=P))
            gam_sb = sing.tile([OL, 1], fp32)
            nc.sync.dma_start(out=gam_sb[:], in_=gam)
            bet_sb = sing.tile([OL, 1], fp32)
            nc.sync.dma_start(out=bet_sb[:], in_=bet)
            fc2b_1p = sing.tile([1, H], fp32)
            nc.sync.dma_start(out=fc2b_1p[:], in_=fc2b)
            fc2b_rep = sing.tile([BL, H], fp32)
            nc.gpsimd.partition_broadcast(fc2b_rep[:], fc2b_1p[:])

            scoresT_all = sing.tile([BL, NT, P], fp32)   # (8, 16, 128) = (8, 2048)
            m_neg_all = sing.tile([BL, NT], fp32)
            partials = sing.tile([P, NT, NC_CHUNKS], fp32)  # (128, 16, 64)

            # ---- phase 1: attention over 16 s-tiles ----
            with ExitStack() as est1:
                ctxp = est1.enter_context(tc.tile_pool(name="ctxp", bufs=2))
                ctx16p = est1.enter_context(tc.tile_pool(name="ctx16p", bufs=2))
                scrp = est1.enter_context(tc.tile_pool(name="scrp", bufs=2))
                smp = est1.enter_context(tc.tile_pool(name="smp", bufs=3))
                pst = est1.enter_context(tc.tile_pool(name="pst", bufs=2, space="PSUM"))
                psw = est1.enter_context(tc.tile_pool(name="psw", bufs=2, space="PSUM"))
                psa = est1.enter_context(tc.tile_pool(name="psa", bufs=2, space="PSUM"))

                for t in range(NT):
                    T = ctxp.tile([P, BL, H], fp32, name=f"T{t}", tag="T")
                    nc.sync.dma_start(out=T[:], in_=ctx_s[t * P:(t + 1) * P, :, :])
                    Tf = T[:].rearrange("p b h -> p (b h)")
                    # fp16 copy for the PE weighted-sum matmuls
                    Th = ctx16p.tile([P, F], fp16, name=f"Th{t}", tag="Th")
                    nc.scalar.copy(out=Th[:], in_=Tf)
                    # scores: fused multiply + accumulate per batch
                    scores_t = smp.tile([P, BL], fp32, name=f"sc{t}", tag="sc")
                    for b in range(BL):
                        scr = scrp.tile([P, H], fp32, name=f"scr{t}_{b}", tag="scr")
                        nc.vector.scalar_tensor_tensor(
                            out=scr[:], in0=T[:, b, :], scalar=1.0,
                            in1=key_rep[:, b, :],
                            op0=Alu.mult, op1=Alu.mult,
                            accum_out=scores_t[:, b:b + 1])
                    nc.vector.tensor_tensor(out=scores_t[:], in0=scores_t[:],
                                            in1=maskT[:, t, :], op=Alu.add)
                    # transpose scores -> (8, 128) row-major batch layout
                    ps_sc = pst.tile([BL, P], fp32, name=f"pssc{t}", tag="pssc")
                    nc.tensor.transpose(ps_sc[:], scores_t[:], ident[:])
                    nc.scalar.copy(out=scoresT_all[:, t, :], in_=ps_sc[:])
                    # per-tile negated max, exp(score - max)
                    nc.vector.tensor_reduce(out=m_neg_all[:, t:t + 1],
                                            in_=scoresT_all[:, t, :],
                                            axis=mybir.AxisListType.X,
                                            op=Alu.max, negate=True)
                    wT = smp.tile([BL, P], fp32, name=f"wT{t}", tag="wT")
                    nc.scalar.activation(out=wT[:], in_=scoresT_all[:, t, :],
                                         func=Act.Exp,
                                         bias=m_neg_all[:, t:t + 1], scale=1.0)
                    # transpose weights back to (128, 8); cast to fp16
                    ps_w = psw.tile([P, BL], fp32, name=f"psw{t}", tag="psw")
                    nc.tensor.transpose(ps_w[:], wT[:], ident[:BL, :BL])
                    w16 = smp.tile([P, BL], fp16, name=f"w{t}", tag="w")
                    nc.scalar.copy(out=w16[:], in_=ps_w[:])
                    # weighted sum: 64 fp16 chunk matmuls
                    ps_att = psa.tile([P, NC_CHUNKS], fp32, name=f"psa{t}", tag="psa")
                    for fc in range(NC_CHUNKS):
                        b = fc // (H // P)
                        nc.tensor.matmul(ps_att[:, fc:fc + 1],
                                         Th[:, fc * P:(fc + 1) * P],
                                         w16[:, b:b + 1], start=True, stop=True)
                    nc.scalar.copy(out=partials[:, t, :], in_=ps_att[:])

            # ---- phase 2: combine partials, softmax outputs, MLP head ----
            with ExitStack() as est2:
                tl = est2.enter_context(tc.tile_pool(name="tl", bufs=2))
                psv = est2.enter_context(tc.tile_pool(name="psv", bufs=2, space="PSUM"))
                psy = est2.enter_context(tc.tile_pool(name="psy", bufs=1, space="PSUM"))
                psz = est2.enter_context(tc.tile_pool(name="psz", bufs=1, space="PSUM"))

                # global negated max per batch
                M_neg = tl.tile([BL, 1], fp32, name="M_neg")
                nc.vector.tensor_reduce(out=M_neg[:], in_=m_neg_all[:],
                                        axis=mybir.AxisListType.X, op=Alu.min)
                # scales[b, t] = exp(m_t - M)
                scales = tl.tile([BL, NT], fp32, name="scales")
                nc.scalar.activation(out=scales[:], in_=m_neg_all[:],
                                     func=Act.Exp, bias=M_neg[:], scale=-1.0)
                # bounce through DRAM to land (t, b) on a single partition
                # (PE operands must start at partition 0/32/64, so slicing a
                # 16-partition tile per t is not allowed as a matmul rhs)
                scales_d = dram.tile([BL, NT], fp32)
                nc.sync.dma_start(out=scales_d[:], in_=scales[:])
                scales1p = tl.tile([1, NT, BL], fp32, name="scales1p")
                nc.sync.dma_start(out=scales1p[:],
                                  in_=scales_d[:].rearrange("(o b) t -> o t b", o=1))
                srep = tl.tile([1, NT, BL, H // P], fp32, name="srep")
                for hc in range(H // P):
                    nc.vector.tensor_copy(out=srep[:, :, :, hc], in_=scales1p[:])
                # acc = sum_t partials_t * exp(m_t - M)
                acc = tl.tile([P, NC_CHUNKS], fp32, name="accT")
                tmp = tl.tile([P, NC_CHUNKS], fp32, name="tmpT")
                for t in range(NT):
                    ps_v = psv.tile([P, NC_CHUNKS], fp32, name=f"psv{t}", tag="psv")
                    nc.tensor.matmul(ps_v[:], ones1[:],
                                     srep[0:1, t, :, :].rearrange("o b c -> o (b c)"),
                                     start=True, stop=True)
                    if t == 0:
                        nc.vector.tensor_tensor(out=acc[:], in0=partials[:, t, :],
                                                in1=ps_v[:], op=Alu.mult)
                    else:
                        nc.vector.tensor_tensor(out=tmp[:], in0=partials[:, t, :],
                                                in1=ps_v[:], op=Alu.mult)
                        nc.vector.tensor_tensor(out=acc[:], in0=acc[:],
                                                in1=tmp[:], op=Alu.add)

                # At = exp(scores - M) / l ; l accumulated by the exp pass
                At_sb = tl.tile([BL, S], fp32, name="At_sb")
                l_sum = tl.tile([BL, 1], fp32, name="l_sum")
                nc.scalar.activation(out=At_sb[:],
                                     in_=scoresT_all[:].rearrange("b t p -> b (t p)"),
                                     func=Act.Exp, bias=M_neg[:], scale=1.0,
                                     accum_out=l_sum[:])
                rec_l = tl.tile([BL, 1], fp32, name="rec_l")
                nc.vector.reciprocal(out=rec_l[:], in_=l_sum[:])
                nc.vector.tensor_scalar_mul(out=At_sb[:], in0=At_sb[:], scalar1=rec_l[:])
                nc.sync.dma_start(out=at_out, in_=At_sb[:])

                # attn = acc / l  (1/l broadcast to (128, 64) via K=1 matmul)
                rec_d = dram.tile([BL, 1], fp32)
                nc.sync.dma_start(out=rec_d[:], in_=rec_l[:])
                rec1p = tl.tile([1, BL], fp32, name="rec1p")
                nc.sync.dma_start(out=rec1p[:],
                                  in_=rec_d[:].rearrange("(o b) u -> o (b u)", o=1))
                rec64 = tl.tile([1, BL, H // P], fp32, name="rec64")
                for hc in range(H // P):
                    nc.vector.tensor_copy(out=rec64[:, :, hc], in_=rec1p[:])
                ps_rb = psv.tile([P, NC_CHUNKS], fp32, name="ps_rb", tag="psv")
                nc.tensor.matmul(ps_rb[:], ones1[:],
                                 rec64[:].rearrange("o b c -> o (b c)"),
                                 start=True, stop=True)
                attnN = tl.tile([P, NC_CHUNKS], fp32, name="attnN")
                nc.vector.tensor_tensor(out=attnN[:], in0=acc[:], in1=ps_rb[:],
                                        op=Alu.mult)

                # regroup (128, [b*8+kc]) -> (128, [kc, b]) == attn^T k-chunks
                attnT_sb = tl.tile([P, H // P, BL], fp32, name="attnT_sb")
                src = attnN[:]
                regroup = bass.AP(tensor=src.tensor, offset=src.offset,
                                  ap=[list(src.ap[0]), [1, H // P], [H // P, BL]])
                nc.vector.tensor_copy(out=attnT_sb[:], in_=regroup)

                # AllGather attn^T across cores
                attnT_bounce = dram.tile([H, BL], fp32)
                nc.sync.dma_start(out=attnT_bounce[:].rearrange("(kc p) b -> p kc b", p=P),
                                  in_=attnT_sb[:])
                attnT_g = dram.tile([N_CORES, H, BL], fp32, addr_space="Shared")
                nc.gpsimd.collective_compute(
                    "AllGather", Alu.bypass,
                    replica_groups=[list(range(N_CORES))],
                    ins=[attnT_bounce.opt()], outs=[attnT_g.opt()])
                rhsA = tl.tile([P, H // P, N_CORES, BL], fp32, name="rhsA")
                g_ap = attnT_g[:]
                for kc in range(H // P):
                    # [p, c, b] slice of (c, k=kc*128+p, b); strides in elems
                    gather_ap = bass.AP(tensor=g_ap.tensor,
                                        offset=g_ap.offset + kc * P * BL,
                                        ap=[[BL, P], [H * BL, N_CORES], [1, BL]])
                    nc.sync.dma_start(out=rhsA[:, kc, :, :], in_=gather_ap)

                # fc1: y^T (128 features, 64 batches)
                ps_y = psy.tile([OL, B], fp32, name="ps_y")
                for kc in range(H // P):
                    nc.tensor.matmul(ps_y[:], w1sb[:, kc, :],
                                     rhsA[:, kc, :, :].rearrange("p c b -> p (c b)"),
                                     start=(kc == 0), stop=False)
                for kc in range(H // P):
                    nc.tensor.matmul(ps_y[:], w1sb[:, H // P + kc, :],
                                     keyT_sb[:, kc, :],
                                     start=False, stop=(kc == H // P - 1))
                y_sb = tl.tile([OL, B], fp32, name="y_sb")
                nc.scalar.copy(out=y_sb[:], in_=ps_y[:])

                # BatchNorm (batch = free dim; stats fully local)
                stats = tl.tile([OL, 6], fp32, name="stats")
                nc.vector.bn_stats(out=stats[:], in_=y_sb[:])
                mv = tl.tile([OL, 2], fp32, name="mv")
                nc.vector.bn_aggr(out=mv[:], in_=stats[:])
                std = tl.tile([OL, 1], fp32, name="std")
                nc.scalar.activation(out=std[:], in_=mv[:, 1:2], func=Act.Sqrt,
                                     bias=eps_sb[:OL, :], scale=1.0)
                rstd = tl.tile([OL, 1], fp32, name="rstd")
                nc.vector.reciprocal(out=rstd[:], in_=std[:])
                ynorm = tl.tile([OL, B], fp32, name="ynorm")
                nc.vector.tensor_scalar(out=ynorm[:], in0=y_sb[:],
                                        scalar1=mv[:, 0:1], scalar2=rstd[:],
                                        op0=Alu.subtract, op1=Alu.mult)
                nc.vector.tensor_scalar(out=ynorm[:], in0=ynorm[:],
                                        scalar1=gam_sb[:], scalar2=bet_sb[:],
                                        op0=Alu.mult, op1=Alu.add)
                tt = tl.tile([OL, B], fp32, name="tt")
                nc.scalar.activation(out=tt[:], in_=ynorm[:], func=Act.Tanh,
                                     bias=0.0, scale=1.0)

                # fc2 partial: z_part (64, 1024) = tt^T @ w2 slice
                ps_z = psz.tile([B, H], fp32, name="ps_z")
                for j in range(2):
                    nc.tensor.matmul(ps_z[:, j * 512:(j + 1) * 512], tt[:],
                                     w2sb[:, j * 512:(j + 1) * 512],
                                     start=True, stop=True)
                zpart = tl.tile([B, H], fp32, name="zpart")
                nc.scalar.copy(out=zpart[:], in_=ps_z[:])
                zp_bounce = dram.tile([B, H], fp32)
                nc.sync.dma_start(out=zp_bounce[:], in_=zpart[:])
                z_rs = dram.tile([BL, H], fp32)
                nc.gpsimd.collective_compute(
                    "ReduceScatter", Alu.add,
                    replica_groups=[list(range(N_CORES))],
                    ins=[zp_bounce.opt()], outs=[z_rs.opt()])
                z_sb = tl.tile([BL, H], fp32, name="z_sb")
                nc.sync.dma_start(out=z_sb[:], in_=z_rs[:])
                nc.vector.tensor_tensor(out=z_sb[:], in0=z_sb[:],
                                        in1=fc2b_rep[:], op=Alu.add)
                nc.sync.dma_start(out=z_out, in_=z_sb[:])

    nc.compile()
    return nc


def _get_nc():
    if "nc" not in _cache:
        _cache["nc"] = _build()
    return _cache["nc"]


def make_in_maps(ctx, key, mask, fc1_w, fc1_b, bn_gamma, bn_beta, fc2_w, fc2_b):
    """Host-side sharding of full inputs into 8 per-core input maps.

    fc1_b is mathematically cancelled by the training-mode BatchNorm that
    immediately follows fc1 (it shifts every batch row equally, and BN
    subtracts the batch mean), so it is not shipped to the device.
    """
    ctx = np.ascontiguousarray(ctx, dtype=np.float32)
    key = np.ascontiguousarray(key, dtype=np.float32)
    mask = np.ascontiguousarray(mask, dtype=np.float32)
    keyT_f = np.ascontiguousarray(key.T)
    W1 = np.ascontiguousarray(fc1_w.T, dtype=np.float32)   # (2H, H)
    W2 = np.ascontiguousarray(fc2_w.T, dtype=np.float32)   # (H, H)
    fc2b = np.ascontiguousarray(fc2_b.reshape(1, H), dtype=np.float32)
    in_maps = []
    for c in range(N_CORES):
        bs = slice(c * BL, (c + 1) * BL)
        os_ = slice(c * OL, (c + 1) * OL)
        in_maps.append({
            "ctx_s": np.ascontiguousarray(ctx[:, bs, :]),
            "key_l": np.ascontiguousarray(key[bs]),
            "keyT_f": keyT_f,
            "maskT_s": np.ascontiguousarray(mask[bs].T),
            "w1s": np.ascontiguousarray(W1[:, os_]),
            "w2s": np.ascontiguousarray(W2[os_, :]),
            "gam": np.ascontiguousarray(np.asarray(bn_gamma, np.float32)[os_].reshape(OL, 1)),
            "bet": np.ascontiguousarray(np.asarray(bn_beta, np.float32)[os_].reshape(OL, 1)),
            "fc2b": fc2b,
        })
    return in_maps


def assemble(results):
    x = np.concatenate([results[c]["z_out"] for c in range(N_CORES)], axis=0)
    At = np.concatenate([results[c]["at_out"] for c in range(N_CORES)], axis=0)
    return x, At


def kernel(ctx, key, mask, fc1_w, fc1_b, bn_gamma, bn_beta, fc2_w, fc2_b):
    import concourse.bass_utils as bass_utils
    nc = _get_nc()
    in_maps = make_in_maps(ctx, key, mask, fc1_w, fc1_b, bn_gamma, bn_beta,
                           fc2_w, fc2_b)
    res = bass_utils.run_bass_kernel_spmd(nc, in_maps,
                                          core_ids=list(range(N_CORES)))
    return assemble(res.results)
